# revision 6
# baseline (speedup 1.0000x reference)
import sys

if "/opt/trn_rl_repo" not in sys.path:
    sys.path.insert(0, "/opt/trn_rl_repo")

import numpy as np
import ml_dtypes

B = 16
N = 4096
S = 1024
D1 = 64
D2 = 256
OUT = 256
NCORES = 8
BPC = 2          # batches per core
NT = 32          # 128-query tiles per batch
NG = 8           # 4-tile groups per batch
TOTAL_N = B * N  # BN stat count
EPS_BN = 1e-5
EPS_D = 1e-8

# K-stack pairing order for the triple-split distance matmul: hi*hi first so
# the large partial sums cancel early in the PE's fp32 accumulation.
PAIRS = [(0, 0), (0, 1), (1, 0), (1, 1), (0, 2), (2, 0), (1, 2), (2, 1), (2, 2)]

LAST_EXEC_NS = None
_CACHE = {}


def _split3(x):
    h1 = x.astype(ml_dtypes.bfloat16)
    r1 = x - h1.astype(np.float32)
    h2 = r1.astype(ml_dtypes.bfloat16)
    r2 = r1 - h2.astype(np.float32)
    h3 = r2.astype(ml_dtypes.bfloat16)
    return (h1, h2, h3)


def _build_aug(xyz1, xyz2):
    a = np.empty((B, 5, N), np.float32)
    a[:, 0:3] = np.transpose(xyz1, (0, 2, 1))
    a[:, 3] = np.square(xyz1).sum(-1)
    a[:, 4] = 1.0
    bb = np.empty((B, 5, S), np.float32)
    bb[:, 0:3] = 2.0 * np.transpose(xyz2, (0, 2, 1))
    bb[:, 3] = -1.0
    bb[:, 4] = -np.square(xyz2).sum(-1)
    asp = _split3(a)
    bsp = _split3(bb)
    aug1 = np.empty((B, 45, N), ml_dtypes.bfloat16)
    aug2 = np.empty((B, 45, S), ml_dtypes.bfloat16)
    for p, (i, j) in enumerate(PAIRS):
        aug1[:, p * 5:(p + 1) * 5] = asp[i]
        aug2[:, p * 5:(p + 1) * 5] = bsp[j]
    return aug1, aug2


def _emit(tc, io, ctx):
    import concourse.bass as bass
    from concourse import mybir

    nc = tc.nc
    f32 = mybir.dt.float32
    f16 = mybir.dt.float16
    bf16 = mybir.dt.bfloat16
    i16 = mybir.dt.int16
    u16 = mybir.dt.uint16
    AF = mybir.ActivationFunctionType
    OP = mybir.AluOpType
    X = mybir.AxisListType.X
    ts = bass.ts

    aug1, aug2, p1, p2h, w1t, w2t, gamma2, beta2, id16, out = io

    const = ctx.enter_context(tc.tile_pool(name="const", bufs=1))
    id16_sb = const.tile([128, 128], f16, name="id16_sb")
    nc.sync.dma_start(id16_sb, id16)
    w1t_sb = const.tile([D1, OUT], f32, name="w1t_sb")
    nc.sync.dma_start(w1t_sb, w1t)
    w2t_sb = const.tile([128, 2, OUT], f16, name="w2t_sb")
    nc.sync.dma_start(w2t_sb[:, 0, :], w2t[0:128, :])
    nc.sync.dma_start(w2t_sb[:, 1, :], w2t[128:256, :])
    gam_sb = const.tile([128, 2], f32, name="gam_sb")
    nc.sync.dma_start(gam_sb, gamma2)
    bet_sb = const.tile([128, 2], f32, name="bet_sb")
    nc.sync.dma_start(bet_sb, beta2)

    # y stored transposed: [o-partition(128), batch, o-half, n]
    yT_sb = const.tile([128, BPC, 2, N], f16, name="yT_sb")
    # per-group BN stat accumulators: [p, stat(y,y2), o-half, slot]
    acc_all = const.tile([128, 2, 2, BPC * NG], f32, name="acc_all")
    bnp = ctx.enter_context(tc.tile_pool(name="bn", bufs=1))

    from contextlib import ExitStack
    with ExitStack() as c1:
        bload = c1.enter_context(tc.tile_pool(name="bload", bufs=2))
        dpool = c1.enter_context(tc.tile_pool(name="dist", bufs=2, space="PSUM"))
        tkpool = c1.enter_context(tc.tile_pool(name="tk", bufs=2))
        scpool = c1.enter_context(tc.tile_pool(name="sc", bufs=2))
        trpool = c1.enter_context(tc.tile_pool(name="trA", bufs=2, space="PSUM"))
        atpool = c1.enter_context(tc.tile_pool(name="AT", bufs=2))
        ytpool = c1.enter_context(tc.tile_pool(name="yt", bufs=1, space="PSUM"))
        sqpool = c1.enter_context(tc.tile_pool(name="sq", bufs=2))

        for b in range(BPC):
            a1 = bload.tile([45, N], bf16, name="a1")
            nc.sync.dma_start(a1, aug1[b])
            a2 = bload.tile([45, S], bf16, name="a2")
            nc.sync.dma_start(a2, aug2[b])
            p1s = bload.tile([D1, N], f32, name="p1s")
            nc.sync.dma_start(p1s, p1[b])
            p2s = bload.tile([128, 2, S], f16, name="p2s")
            nc.sync.dma_start(p2s[:, 0, :], p2h[b, 0:128])
            nc.sync.dma_start(p2s[:, 1, :], p2h[b, 128:256])
            Psb = bload.tile([128, 8, OUT], f16, name="Psb")

            # P[s, o] = sum_c points2[c, s] * W[o, 64 + c]
            for half in range(2):
                pt = dpool.tile([128, 1024], f32, name="dist")
                for j in range(4):
                    sc = half * 4 + j
                    dst = pt[:, j * 256:(j + 1) * 256]
                    nc.tensor.matmul(dst, p2s[:, 0, ts(sc, 128)], w2t_sb[:, 0, :],
                                     start=True, stop=False)
                    nc.tensor.matmul(dst, p2s[:, 1, ts(sc, 128)], w2t_sb[:, 1, :],
                                     start=False, stop=True)
                    nc.scalar.copy(Psb[:, sc, :], dst)

            # Phase A: distances + top-k straight out of PSUM
            top_all = tkpool.tile([128, NT, 8], f32, name="top_all")
            idx_all = tkpool.tile([128, NT, 8], u16, name="idx_all")
            for t in range(NT):
                nd_ps = dpool.tile([128, 1024], f32, name="dist")
                nc.tensor.matmul(nd_ps[:, 0:512], a1[:, ts(t, 128)], a2[:, 0:512],
                                 start=True, stop=True)
                nc.tensor.matmul(nd_ps[:, 512:1024], a1[:, ts(t, 128)], a2[:, 512:1024],
                                 start=True, stop=True)
                nc.vector.max(top_all[:, t, :], nd_ps)
                nc.vector.max_index(idx_all[:, t, :], top_all[:, t, :], nd_ps)

            # Phase B: interpolation weights
            dpe = tkpool.tile([128, NT, 3], f32, name="dpe")
            nc.gpsimd.tensor_scalar(dpe, top_all[:, :, 0:3], -1.0, EPS_D, OP.mult, OP.add)
            recip = tkpool.tile([128, NT, 3], f32, name="recip")
            nc.vector.reciprocal(recip, dpe)
            sum3 = tkpool.tile([128, NT], f32, name="sum3")
            nc.vector.tensor_reduce(sum3, recip, X, OP.add)
            rsum = tkpool.tile([128, NT], f32, name="rsum")
            nc.vector.reciprocal(rsum, sum3)
            w4 = tkpool.tile([128, NT, 4], f16, name="w4")
            nc.gpsimd.memset(w4, 0.0)
            idx4 = tkpool.tile([128, NT, 4], i16, name="idx4")
            nc.gpsimd.memset(idx4, -1)
            for t in range(NT):
                nc.gpsimd.tensor_scalar_mul(w4[:, t, 0:3], recip[:, t, :], rsum[:, t:t + 1])
            nc.gpsimd.tensor_copy(idx4[:, :, 0:3], idx_all.bitcast(i16)[:, :, 0:3])

            # Phase C: per 4-tile group: scatter -> transpose -> conv -> stats
            for g in range(NG):
                AT_sb = atpool.tile([128, 8, 512], f16, name="AT_sb")
                for j in range(4):
                    t = g * 4 + j
                    A_sb = scpool.tile([128, 1024], f16, name="A_sb")
                    nc.gpsimd.local_scatter(A_sb, w4[:, t, :], idx4[:, t, :], 128, 1024, 4)
                    trp = trpool.tile([128, 8, 128], f16, name="trp")
                    for sc in range(8):
                        nc.tensor.transpose(trp[:, sc, :], A_sb[:, ts(sc, 128)], id16_sb)
                    nc.scalar.copy(AT_sb[:, :, j * 128:(j + 1) * 128], trp)

                yt = ytpool.tile([128, 2, 512], f32, name="yt")
                for oh in range(2):
                    nc.tensor.matmul(yt[:, oh, :], w1t_sb[:, ts(oh, 128)],
                                     p1s[:, ts(g, 512)], start=True, stop=False)
                for sc in range(8):
                    for oh in range(2):
                        nc.tensor.matmul(yt[:, oh, :], Psb[:, sc, ts(oh, 128)],
                                         AT_sb[:, sc, :], start=False, stop=(sc == 7))
                slot = b * NG + g
                for oh in range(2):
                    nc.scalar.activation(yT_sb[:, b, oh, ts(g, 512)], yt[:, oh, :],
                                         AF.Copy,
                                         accum_out=acc_all[:, 0, oh, slot:slot + 1])
                    sq = sqpool.tile([128, 512], f16, name="sq")
                    nc.scalar.activation(sq, yt[:, oh, :], AF.Square,
                                         accum_out=acc_all[:, 1, oh, slot:slot + 1])

        # Phase D: BN stats all-reduce + scale/bias (all [128, 2] per-partition)
        red_sb = bnp.tile([128, 2, 2], f32, name="red_sb")
        nc.vector.tensor_reduce(red_sb, acc_all, X, OP.add)
        dram = c1.enter_context(tc.tile_pool(name="dram", bufs=2, space="DRAM"))
        bin_ = dram.tile([128, 2, 2], f32, name="bin")
        bout = dram.tile([128, 2, 2], f32, name="bout")
        nc.gpsimd.dma_start(bin_[:], red_sb[:])
        nc.gpsimd.collective_compute(
            "AllReduce", OP.add,
            replica_groups=[list(range(NCORES))],
            ins=[bin_.opt()],
            outs=[bout.opt()],
        )
        allst = bnp.tile([128, 2, 2], f32, name="allst")
        nc.gpsimd.dma_start(allst[:], bout[:])

        mean = bnp.tile([128, 2], f32, name="mean")
        nc.vector.tensor_scalar_mul(mean, allst[:, 0, :], 1.0 / TOTAL_N)
        e2 = bnp.tile([128, 2], f32, name="e2")
        nc.vector.tensor_scalar_mul(e2, allst[:, 1, :], 1.0 / TOTAL_N)
        msq = bnp.tile([128, 2], f32, name="msq")
        nc.vector.tensor_tensor(msq, mean, mean, OP.mult)
        var = bnp.tile([128, 2], f32, name="var")
        nc.vector.tensor_sub(var, e2, msq)
        vare = bnp.tile([128, 2], f32, name="vare")
        nc.vector.tensor_scalar_add(vare, var, EPS_BN)
        sd = bnp.tile([128, 2], f32, name="sd")
        nc.scalar.sqrt(sd, vare)
        rstd = bnp.tile([128, 2], f32, name="rstd")
        nc.vector.reciprocal(rstd, sd)
        sv = bnp.tile([128, 2], f32, name="sv")
        nc.vector.tensor_tensor(sv, rstd, gam_sb, OP.mult)
        msv = bnp.tile([128, 2], f32, name="msv")
        nc.vector.tensor_tensor(msv, mean, sv, OP.mult)
        bv = bnp.tile([128, 2], f32, name="bv")
        nc.vector.tensor_sub(bv, bet_sb, msv)

    # Phase E: y already transposed; one wide fused scale+bias+relu per o-half
    with ExitStack() as c2:
        opool = c2.enter_context(tc.tile_pool(name="ob", bufs=2))
        for b in range(BPC):
            for oh in range(2):
                ob = opool.tile([128, N], f32, name="ob")
                nc.scalar.activation(ob, yT_sb[:, b, oh, :], AF.Relu,
                                     bias=bv[:, oh:oh + 1],
                                     scale=sv[:, oh:oh + 1])
                nc.sync.dma_start(out[b, ts(oh, 128), :], ob)


def _get_compiled():
    if "nc" in _CACHE:
        return _CACHE["nc"]
    import concourse.tile as tile
    from concourse import bacc, mybir
    from contextlib import ExitStack

    f32 = mybir.dt.float32
    f16 = mybir.dt.float16
    bf16 = mybir.dt.bfloat16

    nc = bacc.Bacc("TRN2", target_bir_lowering=False, debug=False, num_devices=NCORES)

    def din(name, shape, dt):
        return nc.dram_tensor(name, shape, dt, kind="ExternalInput").ap()

    io = (
        din("aug1", [BPC, 45, N], bf16),
        din("aug2", [BPC, 45, S], bf16),
        din("p1", [BPC, D1, N], f32),
        din("p2h", [BPC, D2, S], f16),
        din("w1t", [D1, OUT], f32),
        din("w2t", [D2, OUT], f16),
        din("gamma2", [128, 2], f32),
        din("beta2", [128, 2], f32),
        din("id16", [128, 128], f16),
        nc.dram_tensor("out", [BPC, OUT, N], f32, kind="ExternalOutput").ap(),
    )
    with tile.TileContext(nc) as tc:
        with ExitStack() as ctx:
            _emit(tc, io, ctx)
    nc.compile()
    _CACHE["nc"] = nc
    return nc


def kernel(_trace=False, **inputs):
    global LAST_EXEC_NS
    from concourse import bass_utils

    xyz1 = np.asarray(inputs["xyz1"], np.float32)
    xyz2 = np.asarray(inputs["xyz2"], np.float32)
    points1 = np.asarray(inputs["points1"], np.float32)
    points2 = np.asarray(inputs["points2"], np.float32)
    W = np.asarray(inputs["W"], np.float32)
    gamma = np.asarray(inputs["gamma"], np.float32).reshape(OUT)
    beta = np.asarray(inputs["beta"], np.float32).reshape(OUT)

    aug1, aug2 = _build_aug(xyz1, xyz2)
    w1t = np.ascontiguousarray(W[:, 0:D1].T)
    w2t = np.ascontiguousarray(W[:, D1:].T).astype(np.float16)
    p2h = points2.astype(np.float16)
    id16 = np.eye(128, dtype=np.float16)
    gamma2 = np.ascontiguousarray(gamma.reshape(2, 128).T)
    beta2 = np.ascontiguousarray(beta.reshape(2, 128).T)

    nc = _get_compiled()
    in_maps = []
    for k in range(NCORES):
        sl = slice(k * BPC, (k + 1) * BPC)
        in_maps.append({
            "aug1": np.ascontiguousarray(aug1[sl]),
            "aug2": np.ascontiguousarray(aug2[sl]),
            "p1": np.ascontiguousarray(points1[sl]),
            "p2h": np.ascontiguousarray(p2h[sl]),
            "w1t": w1t,
            "w2t": w2t,
            "gamma2": gamma2,
            "beta2": beta2,
            "id16": id16,
        })

    br = bass_utils.run_bass_kernel_spmd(nc, in_maps, list(range(NCORES)), trace=_trace)
    LAST_EXEC_NS = br.exec_time_ns

    full = np.empty((B, OUT, N), np.float32)
    for k in range(NCORES):
        rk = br.results[k]
        o = rk["out"] if isinstance(rk, dict) else rk[0]
        full[k * BPC:(k + 1) * BPC] = np.asarray(o, np.float32).reshape(BPC, OUT, N)
    return full


# revision 7
# speedup vs baseline: 1.2825x; 1.2825x over previous
import sys

if "/opt/trn_rl_repo" not in sys.path:
    sys.path.insert(0, "/opt/trn_rl_repo")

import numpy as np
import ml_dtypes

B = 16
N = 4096
S = 1024
D1 = 64
D2 = 256
OUT = 256
NCORES = 8
BPC = 2          # batches per core
NT = 32          # 128-query tiles per batch
NG = 8           # 4-tile groups per batch
TOTAL_N = B * N  # BN stat count
EPS_BN = 1e-5
EPS_D = 1e-8

# K-stack pairing order for the triple-split distance matmul: hi*hi first so
# the large partial sums cancel early in the PE's fp32 accumulation.
PAIRS = [(0, 0), (0, 1), (1, 0), (1, 1), (0, 2), (2, 0), (1, 2), (2, 1), (2, 2)]

LAST_EXEC_NS = None
_CACHE = {}


def _split3(x):
    h1 = x.astype(ml_dtypes.bfloat16)
    r1 = x - h1.astype(np.float32)
    h2 = r1.astype(ml_dtypes.bfloat16)
    r2 = r1 - h2.astype(np.float32)
    h3 = r2.astype(ml_dtypes.bfloat16)
    return (h1, h2, h3)


def _build_aug(xyz1, xyz2):
    a = np.empty((B, 5, N), np.float32)
    a[:, 0:3] = np.transpose(xyz1, (0, 2, 1))
    a[:, 3] = np.square(xyz1).sum(-1)
    a[:, 4] = 1.0
    bb = np.empty((B, 5, S), np.float32)
    bb[:, 0:3] = 2.0 * np.transpose(xyz2, (0, 2, 1))
    bb[:, 3] = -1.0
    bb[:, 4] = -np.square(xyz2).sum(-1)
    asp = _split3(a)
    bsp = _split3(bb)
    aug1 = np.empty((B, 45, N), ml_dtypes.bfloat16)
    aug2 = np.empty((B, 45, S), ml_dtypes.bfloat16)
    for p, (i, j) in enumerate(PAIRS):
        aug1[:, p * 5:(p + 1) * 5] = asp[i]
        aug2[:, p * 5:(p + 1) * 5] = bsp[j]
    return aug1, aug2


def _emit(tc, io, ctx):
    import concourse.bass as bass
    from concourse import mybir

    nc = tc.nc
    f32 = mybir.dt.float32
    f16 = mybir.dt.float16
    bf16 = mybir.dt.bfloat16
    i16 = mybir.dt.int16
    u16 = mybir.dt.uint16
    AF = mybir.ActivationFunctionType
    OP = mybir.AluOpType
    X = mybir.AxisListType.X
    ts = bass.ts

    aug1, aug2, p1, p2h, w1t, w2t, gamma2, beta2, id16, out = io

    const = ctx.enter_context(tc.tile_pool(name="const", bufs=1))
    id16_sb = const.tile([128, 128], f16, name="id16_sb")
    nc.sync.dma_start(id16_sb, id16)
    w1t_sb = const.tile([D1, OUT], f32, name="w1t_sb")
    nc.sync.dma_start(w1t_sb, w1t)
    w2t_sb = const.tile([128, 2, OUT], f16, name="w2t_sb")
    nc.sync.dma_start(w2t_sb[:, 0, :], w2t[0:128, :])
    nc.sync.dma_start(w2t_sb[:, 1, :], w2t[128:256, :])
    gam_sb = const.tile([128, 2], f32, name="gam_sb")
    nc.sync.dma_start(gam_sb, gamma2)
    bet_sb = const.tile([128, 2], f32, name="bet_sb")
    nc.sync.dma_start(bet_sb, beta2)

    # y stored transposed: [o-partition(128), batch, o-half, n]
    yT_sb = const.tile([128, BPC, 2, N], f16, name="yT_sb")
    # per-group BN stat accumulators: [p, stat(y,y2), o-half, slot]
    acc_all = const.tile([128, 2, 2, BPC * NG], f32, name="acc_all")
    bnp = ctx.enter_context(tc.tile_pool(name="bn", bufs=1))

    from contextlib import ExitStack
    with ExitStack() as c1:
        bload = c1.enter_context(tc.tile_pool(name="bload", bufs=2))
        dpool = c1.enter_context(tc.tile_pool(name="dist", bufs=2, space="PSUM"))
        tkpool = c1.enter_context(tc.tile_pool(name="tk", bufs=2))
        bsm = c1.enter_context(tc.tile_pool(name="bsm", bufs=2))
        scpool = c1.enter_context(tc.tile_pool(name="sc", bufs=2))
        trpool = c1.enter_context(tc.tile_pool(name="trA", bufs=2, space="PSUM"))
        atpool = c1.enter_context(tc.tile_pool(name="AT", bufs=2))
        ytpool = c1.enter_context(tc.tile_pool(name="yt", bufs=1, space="PSUM"))
        sqpool = c1.enter_context(tc.tile_pool(name="sq", bufs=2))

        st = {}

        def prologue(b):
            a1 = bload.tile([45, N], bf16, name="a1")
            nc.sync.dma_start(a1, aug1[b])
            a2 = bload.tile([45, S], bf16, name="a2")
            nc.sync.dma_start(a2, aug2[b])
            p2s = bload.tile([128, 2, S], f16, name="p2s")
            nc.sync.dma_start(p2s[:, 0, :], p2h[b, 0:128])
            nc.sync.dma_start(p2s[:, 1, :], p2h[b, 128:256])
            Psb = bload.tile([128, 8, OUT], f16, name="Psb")
            # P[s, o] = sum_c points2[c, s] * W[o, 64 + c]
            for half in range(2):
                pt = dpool.tile([128, 1024], f32, name="dist")
                for j in range(4):
                    sc = half * 4 + j
                    dst = pt[:, j * 256:(j + 1) * 256]
                    nc.tensor.matmul(dst, p2s[:, 0, ts(sc, 128)], w2t_sb[:, 0, :],
                                     start=True, stop=False)
                    nc.tensor.matmul(dst, p2s[:, 1, ts(sc, 128)], w2t_sb[:, 1, :],
                                     start=False, stop=True)
                    nc.scalar.copy(Psb[:, sc, :], dst)
            p1s = bload.tile([D1, N], f32, name="p1s")
            nc.sync.dma_start(p1s, p1[b])
            top_all = tkpool.tile([128, NT, 8], f32, name="top_all")
            idx_all = tkpool.tile([128, NT, 8], u16, name="idx_all")
            w4 = tkpool.tile([128, NT, 4], f16, name="w4")
            nc.gpsimd.memset(w4, 0.0)
            idx4 = tkpool.tile([128, NT, 4], i16, name="idx4")
            nc.gpsimd.memset(idx4, -1)
            st[b] = (a1, a2, p1s, p2s, Psb, top_all, idx_all, w4, idx4)

        def phaseA(b, g):
            a1, a2, p1s, p2s, Psb, top_all, idx_all, w4, idx4 = st[b]
            for j in range(4):
                t = g * 4 + j
                nd_ps = dpool.tile([128, 1024], f32, name="dist")
                nc.tensor.matmul(nd_ps[:, 0:512], a1[:, ts(t, 128)], a2[:, 0:512],
                                 start=True, stop=True)
                nc.tensor.matmul(nd_ps[:, 512:1024], a1[:, ts(t, 128)], a2[:, 512:1024],
                                 start=True, stop=True)
                nc.vector.max(top_all[:, t, :], nd_ps)
                nc.vector.max_index(idx_all[:, t, :], top_all[:, t, :], nd_ps)

        def phaseB(b, g):
            a1, a2, p1s, p2s, Psb, top_all, idx_all, w4, idx4 = st[b]
            sl = slice(g * 4, g * 4 + 4)
            dpe = bsm.tile([128, 4, 3], f32, name="dpe")
            nc.gpsimd.tensor_scalar(dpe, top_all[:, sl, 0:3], -1.0, EPS_D, OP.mult, OP.add)
            recip = bsm.tile([128, 4, 3], f32, name="recip")
            nc.vector.reciprocal(recip, dpe)
            sum3 = bsm.tile([128, 4], f32, name="sum3")
            nc.vector.tensor_reduce(sum3, recip, X, OP.add)
            rsum = bsm.tile([128, 4], f32, name="rsum")
            nc.vector.reciprocal(rsum, sum3)
            for j in range(4):
                t = g * 4 + j
                nc.gpsimd.tensor_scalar_mul(w4[:, t, 0:3], recip[:, j, :], rsum[:, j:j + 1])
            nc.gpsimd.tensor_copy(idx4[:, sl, 0:3], idx_all.bitcast(i16)[:, sl, 0:3])

        def phaseC(b, g):
            a1, a2, p1s, p2s, Psb, top_all, idx_all, w4, idx4 = st[b]
            AT_sb = atpool.tile([128, 8, 512], f16, name="AT_sb")
            for j in range(4):
                t = g * 4 + j
                A_sb = scpool.tile([128, 1024], f16, name="A_sb")
                nc.gpsimd.local_scatter(A_sb, w4[:, t, :], idx4[:, t, :], 128, 1024, 4)
                trp = trpool.tile([128, 8, 128], f16, name="trp")
                for sc in range(8):
                    nc.tensor.transpose(trp[:, sc, :], A_sb[:, ts(sc, 128)], id16_sb)
                nc.scalar.copy(AT_sb[:, :, j * 128:(j + 1) * 128], trp)

            yt = ytpool.tile([128, 2, 512], f32, name="yt")
            for oh in range(2):
                nc.tensor.matmul(yt[:, oh, :], w1t_sb[:, ts(oh, 128)],
                                 p1s[:, ts(g, 512)], start=True, stop=False)
            for sc in range(8):
                for oh in range(2):
                    nc.tensor.matmul(yt[:, oh, :], Psb[:, sc, ts(oh, 128)],
                                     AT_sb[:, sc, :], start=False, stop=(sc == 7))
            slot = b * NG + g
            for oh in range(2):
                nc.scalar.activation(yT_sb[:, b, oh, ts(g, 512)], yt[:, oh, :],
                                     AF.Copy,
                                     accum_out=acc_all[:, 0, oh, slot:slot + 1])
                sq = sqpool.tile([128, 512], f16, name="sq")
                nc.scalar.activation(sq, yt[:, oh, :], AF.Square,
                                     accum_out=acc_all[:, 1, oh, slot:slot + 1])

        # software pipeline: A/B of group i+1 overlap C of group i
        prev = None
        for b in range(BPC):
            for g in range(NG):
                if g == 0:
                    prologue(b)
                phaseA(b, g)
                if prev is not None:
                    phaseC(*prev)
                phaseB(b, g)
                prev = (b, g)
        phaseC(*prev)

        # Phase D: BN stats all-reduce + scale/bias (all [128, 2] per-partition)
        red_sb = bnp.tile([128, 2, 2], f32, name="red_sb")
        nc.vector.tensor_reduce(red_sb, acc_all, X, OP.add)
        dram = c1.enter_context(tc.tile_pool(name="dram", bufs=2, space="DRAM"))
        bin_ = dram.tile([128, 2, 2], f32, name="bin")
        bout = dram.tile([128, 2, 2], f32, name="bout")
        nc.gpsimd.dma_start(bin_[:], red_sb[:])
        nc.gpsimd.collective_compute(
            "AllReduce", OP.add,
            replica_groups=[list(range(NCORES))],
            ins=[bin_.opt()],
            outs=[bout.opt()],
        )
        allst = bnp.tile([128, 2, 2], f32, name="allst")
        nc.gpsimd.dma_start(allst[:], bout[:])

        mean = bnp.tile([128, 2], f32, name="mean")
        nc.vector.tensor_scalar_mul(mean, allst[:, 0, :], 1.0 / TOTAL_N)
        e2 = bnp.tile([128, 2], f32, name="e2")
        nc.vector.tensor_scalar_mul(e2, allst[:, 1, :], 1.0 / TOTAL_N)
        msq = bnp.tile([128, 2], f32, name="msq")
        nc.vector.tensor_tensor(msq, mean, mean, OP.mult)
        var = bnp.tile([128, 2], f32, name="var")
        nc.vector.tensor_sub(var, e2, msq)
        vare = bnp.tile([128, 2], f32, name="vare")
        nc.vector.tensor_scalar_add(vare, var, EPS_BN)
        sd = bnp.tile([128, 2], f32, name="sd")
        nc.scalar.sqrt(sd, vare)
        rstd = bnp.tile([128, 2], f32, name="rstd")
        nc.vector.reciprocal(rstd, sd)
        sv = bnp.tile([128, 2], f32, name="sv")
        nc.vector.tensor_tensor(sv, rstd, gam_sb, OP.mult)
        msv = bnp.tile([128, 2], f32, name="msv")
        nc.vector.tensor_tensor(msv, mean, sv, OP.mult)
        bv = bnp.tile([128, 2], f32, name="bv")
        nc.vector.tensor_sub(bv, bet_sb, msv)

    # Phase E: y already transposed; one wide fused scale+bias+relu per o-half
    with ExitStack() as c2:
        opool = c2.enter_context(tc.tile_pool(name="ob", bufs=2))
        for b in range(BPC):
            for oh in range(2):
                ob = opool.tile([128, N], f32, name="ob")
                nc.scalar.activation(ob, yT_sb[:, b, oh, :], AF.Relu,
                                     bias=bv[:, oh:oh + 1],
                                     scale=sv[:, oh:oh + 1])
                nc.sync.dma_start(out[b, ts(oh, 128), :], ob)


def _get_compiled():
    if "nc" in _CACHE:
        return _CACHE["nc"]
    import concourse.tile as tile
    from concourse import bacc, mybir
    from contextlib import ExitStack

    f32 = mybir.dt.float32
    f16 = mybir.dt.float16
    bf16 = mybir.dt.bfloat16

    nc = bacc.Bacc("TRN2", target_bir_lowering=False, debug=False, num_devices=NCORES)

    def din(name, shape, dt):
        return nc.dram_tensor(name, shape, dt, kind="ExternalInput").ap()

    io = (
        din("aug1", [BPC, 45, N], bf16),
        din("aug2", [BPC, 45, S], bf16),
        din("p1", [BPC, D1, N], f32),
        din("p2h", [BPC, D2, S], f16),
        din("w1t", [D1, OUT], f32),
        din("w2t", [D2, OUT], f16),
        din("gamma2", [128, 2], f32),
        din("beta2", [128, 2], f32),
        din("id16", [128, 128], f16),
        nc.dram_tensor("out", [BPC, OUT, N], f32, kind="ExternalOutput").ap(),
    )
    with tile.TileContext(nc) as tc:
        with ExitStack() as ctx:
            _emit(tc, io, ctx)
    nc.compile()
    _CACHE["nc"] = nc
    return nc


def kernel(_trace=False, **inputs):
    global LAST_EXEC_NS
    from concourse import bass_utils

    xyz1 = np.asarray(inputs["xyz1"], np.float32)
    xyz2 = np.asarray(inputs["xyz2"], np.float32)
    points1 = np.asarray(inputs["points1"], np.float32)
    points2 = np.asarray(inputs["points2"], np.float32)
    W = np.asarray(inputs["W"], np.float32)
    gamma = np.asarray(inputs["gamma"], np.float32).reshape(OUT)
    beta = np.asarray(inputs["beta"], np.float32).reshape(OUT)

    aug1, aug2 = _build_aug(xyz1, xyz2)
    w1t = np.ascontiguousarray(W[:, 0:D1].T)
    w2t = np.ascontiguousarray(W[:, D1:].T).astype(np.float16)
    p2h = points2.astype(np.float16)
    id16 = np.eye(128, dtype=np.float16)
    gamma2 = np.ascontiguousarray(gamma.reshape(2, 128).T)
    beta2 = np.ascontiguousarray(beta.reshape(2, 128).T)

    nc = _get_compiled()
    in_maps = []
    for k in range(NCORES):
        sl = slice(k * BPC, (k + 1) * BPC)
        in_maps.append({
            "aug1": np.ascontiguousarray(aug1[sl]),
            "aug2": np.ascontiguousarray(aug2[sl]),
            "p1": np.ascontiguousarray(points1[sl]),
            "p2h": np.ascontiguousarray(p2h[sl]),
            "w1t": w1t,
            "w2t": w2t,
            "gamma2": gamma2,
            "beta2": beta2,
            "id16": id16,
        })

    br = bass_utils.run_bass_kernel_spmd(nc, in_maps, list(range(NCORES)), trace=_trace)
    LAST_EXEC_NS = br.exec_time_ns

    full = np.empty((B, OUT, N), np.float32)
    for k in range(NCORES):
        rk = br.results[k]
        o = rk["out"] if isinstance(rk, dict) else rk[0]
        full[k * BPC:(k + 1) * BPC] = np.asarray(o, np.float32).reshape(BPC, OUT, N)
    return full
